# revision 2
# baseline (speedup 1.0000x reference)
"""Trainium2 Bass kernel: per-channel exponential moving average.

  a_t = k*x_t + (1-k)*a_{t-1},  a_{-1} = x_0   (per batch, per channel)

Full inputs: x [16, 8000, 512] f32, smooth [512] f32. Output [16, 8000, 512].

Strategy (8 NeuronCores, data-parallel over batch, 2 batches/core):
  - Host pre-scales kx = k*x AND pre-transposes it to [B_LOC, C, T] so
    channels land on SBUF partitions and time is the free axis. The device
    kernel is then pure DMA + DVE scan: no PE transposes, no PSUM, no
    ACT copies (the old PE-transpose pipeline was Tensor-engine-bound at
    ~376us of fp32 transposes).
  - SWDGE (gpsimd) DMA for all bulk traffic (sprays 16 SDMA engines);
    per-partition descriptors are TC*dsize contiguous bytes.
  - DVE tensor_tensor_scan runs state = d*state + kx along time, chained
    chunk-to-chunk via the previous output tile's last column.
  - Streams = (batch, channel-group): 8 independent chains per core,
    processed stream-major so the carry is always the just-produced tile.
  - Host transposes the output back to [B, T, C].
"""
import numpy as np
from contextlib import ExitStack

import concourse.bass as bass
from concourse import bacc, mybir
import concourse.tile as tile
from concourse.bass_utils import run_bass_kernel_spmd

B, T, C = 16, 8000, 512
NCORES = 8
B_LOC = B // NCORES  # batches per core
P = 128
CG = C // P          # channel groups
TC = 2000            # time chunk (free dim per scan)
NCH = T // TC        # chunks per stream
F32 = mybir.dt.float32
BF16 = mybir.dt.bfloat16

# I/O dtype switches (scan state is always fp32 internally)
IN_BF16 = False
OUT_BF16 = False
# streams whose scans run on gpsimd instead of DVE (stream = b*CG + cg)
GP_STREAMS = ()

_CACHED_NC = None


def _build_nc():
    in_dt = BF16 if IN_BF16 else F32
    out_dt = BF16 if OUT_BF16 else F32

    nc = bacc.Bacc(None, target_bir_lowering=False)
    x = nc.declare_dram_parameter("x", [B_LOC, C, T], in_dt, isOutput=False)
    d_pc = nc.declare_dram_parameter("d_pc", [P, CG], F32, isOutput=False)
    x0t = nc.declare_dram_parameter("x0t", [P, CG, B_LOC], F32, isOutput=False)
    y = nc.declare_dram_parameter("y", [B_LOC, C, T], out_dt, isOutput=True)

    with tile.TileContext(nc) as tc, ExitStack() as ctx:
        singles = ctx.enter_context(tc.tile_pool(name="singles", bufs=1))
        inpool = ctx.enter_context(tc.tile_pool(name="inpool", bufs=6))
        outpool = ctx.enter_context(tc.tile_pool(name="outpool", bufs=6))

        d_sb = singles.tile([P, CG], F32)
        nc.sync.dma_start(out=d_sb[:], in_=d_pc[:])
        x0_sb = singles.tile([P, CG, B_LOC], F32)
        nc.sync.dma_start(out=x0_sb[:], in_=x0t[:])
        ones = singles.tile([P, TC], F32)
        nc.vector.memset(ones[:], 1.0)
        d_bc = singles.tile([P, CG, TC], F32)
        for cg in range(CG):
            nc.scalar.activation(
                d_bc[:, cg, :], ones[:],
                mybir.ActivationFunctionType.Copy,
                scale=d_sb[:, cg : cg + 1],
            )

        for b in range(B_LOC):
            for cg in range(CG):
                s = b * CG + cg
                eng = nc.gpsimd if s in GP_STREAMS else nc.vector
                prev = None
                for ch in range(NCH):
                    t0 = ch * TC
                    xin = inpool.tile([P, TC], in_dt, tag="xin", name="xin")
                    nc.gpsimd.dma_start(
                        out=xin[:],
                        in_=x[b, cg * P : (cg + 1) * P, t0 : t0 + TC],
                    )
                    so = outpool.tile([P, TC], out_dt, tag="so", name="so")
                    init = (
                        x0_sb[:, cg, b : b + 1]
                        if ch == 0
                        else prev[:, TC - 1 : TC]
                    )
                    eng.tensor_tensor_scan(
                        so[:],
                        d_bc[:, cg, :],
                        xin[:],
                        init,
                        mybir.AluOpType.mult,
                        mybir.AluOpType.add,
                    )
                    prev = so
                    nc.gpsimd.dma_start(
                        out=y[b, cg * P : (cg + 1) * P, t0 : t0 + TC],
                        in_=so[:],
                    )
    nc.compile()
    return nc


def _get_nc():
    global _CACHED_NC
    if _CACHED_NC is None:
        _CACHED_NC = _build_nc()
    return _CACHED_NC


def _prep_in_maps(inputs, smooth):
    x = np.asarray(inputs, dtype=np.float32)
    sm = np.asarray(smooth, dtype=np.float32)
    k = np.clip(sm, 0.0, 1.0).astype(np.float32)
    d = (1.0 - k).astype(np.float32)
    # [B, C, T], channels on partitions, time contiguous
    kxT = np.ascontiguousarray((x * k[None, None, :]).transpose(0, 2, 1))
    if IN_BF16:
        import ml_dtypes

        kxT = kxT.astype(ml_dtypes.bfloat16)
    d_pc = np.ascontiguousarray(d.reshape(CG, P).T)
    # raw x[:, 0, :] transposed: x0t[p, g, b] = x[b, 0, g*P + p]
    nb = x.shape[0]
    x0t = np.ascontiguousarray(x[:, 0, :].T.reshape(CG, P, nb).transpose(1, 0, 2))
    return [
        {
            "x": np.ascontiguousarray(kxT[i * B_LOC : (i + 1) * B_LOC]),
            "d_pc": d_pc,
            "x0t": np.ascontiguousarray(x0t[:, :, i * B_LOC : (i + 1) * B_LOC]),
        }
        for i in range(NCORES)
    ]


def _install_ntff_shim():
    """Provide antenv.axon_hooks if the image lacks it (trace=True path).

    Replicates trn_agent_boot's ctypes NTFF hook against libaxon_pjrt.so.
    """
    import sys

    if "antenv.axon_hooks" in sys.modules:
        return
    try:
        import antenv.axon_hooks  # noqa: F401
        return
    except ImportError:
        pass
    import contextlib
    import ctypes
    import types

    so_path = "/opt/axon/libaxon_pjrt.so"
    try:
        lib = ctypes.CDLL(so_path)
    except OSError:
        return
    if not hasattr(lib, "axon_start_nrt_profile"):
        return
    lib.axon_start_nrt_profile.argtypes = [
        ctypes.POINTER(ctypes.c_int64),
        ctypes.c_size_t,
    ]
    lib.axon_start_nrt_profile.restype = ctypes.c_int64
    lib.axon_stop_nrt_profile.argtypes = [ctypes.c_char_p]
    lib.axon_stop_nrt_profile.restype = ctypes.c_int64

    @contextlib.contextmanager
    def _hook(output_dir, device_ids):
        import jax

        jax.devices()
        if device_ids:
            ids = (ctypes.c_int64 * len(device_ids))(*device_ids)
            rc = lib.axon_start_nrt_profile(ids, len(device_ids))
        else:
            rc = lib.axon_start_nrt_profile(None, 0)
        if rc != 0:
            raise RuntimeError(f"axon_start_nrt_profile rc={rc}")
        try:
            yield
        finally:
            n = lib.axon_stop_nrt_profile(str(output_dir).encode())
            print(f"ntff profile: {n} file(s) written to {output_dir}")

    mod = types.ModuleType("antenv.axon_hooks")
    mod.get_axon_ntff_profile_hook = lambda: _hook
    mod.set_axon_ntff_profile_hook = lambda h: None
    sys.modules["antenv.axon_hooks"] = mod


def run(inputs, smooth, trace=False, **trace_kwargs):
    """Run on 8 cores; returns (y_full, BassKernelResults)."""
    if trace:
        _install_ntff_shim()
    nc = _get_nc()
    in_maps = _prep_in_maps(inputs, smooth)
    res = run_bass_kernel_spmd(
        nc, in_maps, list(range(NCORES)), trace=trace, **trace_kwargs
    )
    yT = np.concatenate([res.results[i]["y"] for i in range(NCORES)], axis=0)
    y = np.ascontiguousarray(yT.astype(np.float32).transpose(0, 2, 1))
    return y, res


def kernel(inputs, smooth):
    y, _ = run(inputs, smooth)
    return y


# revision 4
# speedup vs baseline: 1.4701x; 1.4701x over previous
"""Trainium2 Bass kernel: per-channel exponential moving average.

  a_t = k*x_t + (1-k)*a_{t-1},  a_{-1} = x_0   (per batch, per channel)

Full inputs: x [16, 8000, 512] f32, smooth [512] f32. Output [16, 8000, 512].

Strategy (8 NeuronCores, data-parallel over batch, 2 batches/core):
  - Host pre-scales kx = k*x AND pre-transposes it to [B_LOC, C, T] so
    channels land on SBUF partitions and time is the free axis. The device
    kernel is then pure DMA + DVE scan: no PE transposes, no PSUM, no
    ACT copies (the old PE-transpose pipeline was Tensor-engine-bound at
    ~376us of fp32 transposes).
  - SWDGE (gpsimd) DMA for all bulk traffic (sprays 16 SDMA engines);
    per-partition descriptors are TC*dsize contiguous bytes.
  - DVE tensor_tensor_scan runs state = d*state + kx along time, chained
    chunk-to-chunk via the previous output tile's last column.
  - Streams = (batch, channel-group): 8 independent chains per core,
    processed stream-major so the carry is always the just-produced tile.
  - Host transposes the output back to [B, T, C].
"""
import numpy as np
from contextlib import ExitStack

import concourse.bass as bass
from concourse import bacc, mybir
import concourse.tile as tile
from concourse.bass_utils import run_bass_kernel_spmd

B, T, C = 16, 8000, 512
NCORES = 8
B_LOC = B // NCORES  # batches per core
P = 128
CG = C // P          # channel groups
TC = 2000            # time chunk (free dim per scan)
NCH = T // TC        # chunks per stream
F32 = mybir.dt.float32
BF16 = mybir.dt.bfloat16

# I/O dtype switches (scan state is always fp32 internally)
IN_BF16 = True
OUT_BF16 = True
# streams whose scans run on gpsimd instead of DVE (stream = b*CG + cg)
GP_STREAMS = ()

_CACHED_NC = None


def _build_nc():
    in_dt = BF16 if IN_BF16 else F32
    out_dt = BF16 if OUT_BF16 else F32

    nc = bacc.Bacc(None, target_bir_lowering=False)
    x = nc.declare_dram_parameter("x", [B_LOC, C, T], in_dt, isOutput=False)
    d_pc = nc.declare_dram_parameter("d_pc", [P, CG], F32, isOutput=False)
    x0t = nc.declare_dram_parameter("x0t", [P, CG, B_LOC], F32, isOutput=False)
    y = nc.declare_dram_parameter("y", [B_LOC, C, T], out_dt, isOutput=True)

    with tile.TileContext(nc) as tc, ExitStack() as ctx:
        singles = ctx.enter_context(tc.tile_pool(name="singles", bufs=1))
        inpool = ctx.enter_context(tc.tile_pool(name="inpool", bufs=6))
        outpool = ctx.enter_context(tc.tile_pool(name="outpool", bufs=6))

        d_sb = singles.tile([P, CG], F32)
        nc.sync.dma_start(out=d_sb[:], in_=d_pc[:])
        x0_sb = singles.tile([P, CG, B_LOC], F32)
        nc.sync.dma_start(out=x0_sb[:], in_=x0t[:])
        ones = singles.tile([P, TC], F32)
        nc.vector.memset(ones[:], 1.0)
        d_bc = singles.tile([P, CG, TC], F32)
        for cg in range(CG):
            nc.scalar.activation(
                d_bc[:, cg, :], ones[:],
                mybir.ActivationFunctionType.Copy,
                scale=d_sb[:, cg : cg + 1],
            )

        for b in range(B_LOC):
            for cg in range(CG):
                s = b * CG + cg
                eng = nc.gpsimd if s in GP_STREAMS else nc.vector
                prev = None
                for ch in range(NCH):
                    t0 = ch * TC
                    xin = inpool.tile([P, TC], in_dt, tag="xin", name="xin")
                    nc.gpsimd.dma_start(
                        out=xin[:],
                        in_=x[b, cg * P : (cg + 1) * P, t0 : t0 + TC],
                    )
                    so = outpool.tile([P, TC], out_dt, tag="so", name="so")
                    init = (
                        x0_sb[:, cg, b : b + 1]
                        if ch == 0
                        else prev[:, TC - 1 : TC]
                    )
                    eng.tensor_tensor_scan(
                        so[:],
                        d_bc[:, cg, :],
                        xin[:],
                        init,
                        mybir.AluOpType.mult,
                        mybir.AluOpType.add,
                    )
                    prev = so
                    nc.gpsimd.dma_start(
                        out=y[b, cg * P : (cg + 1) * P, t0 : t0 + TC],
                        in_=so[:],
                    )
    nc.compile()
    return nc


def _get_nc():
    global _CACHED_NC
    if _CACHED_NC is None:
        _CACHED_NC = _build_nc()
    return _CACHED_NC


def _prep_in_maps(inputs, smooth):
    x = np.asarray(inputs, dtype=np.float32)
    sm = np.asarray(smooth, dtype=np.float32)
    k = np.clip(sm, 0.0, 1.0).astype(np.float32)
    d = (1.0 - k).astype(np.float32)
    # [B, C, T], channels on partitions, time contiguous
    kxT = np.ascontiguousarray((x * k[None, None, :]).transpose(0, 2, 1))
    if IN_BF16:
        import ml_dtypes

        kxT = kxT.astype(ml_dtypes.bfloat16)
    d_pc = np.ascontiguousarray(d.reshape(CG, P).T)
    # raw x[:, 0, :] transposed: x0t[p, g, b] = x[b, 0, g*P + p]
    nb = x.shape[0]
    x0t = np.ascontiguousarray(x[:, 0, :].T.reshape(CG, P, nb).transpose(1, 0, 2))
    return [
        {
            "x": np.ascontiguousarray(kxT[i * B_LOC : (i + 1) * B_LOC]),
            "d_pc": d_pc,
            "x0t": np.ascontiguousarray(x0t[:, :, i * B_LOC : (i + 1) * B_LOC]),
        }
        for i in range(NCORES)
    ]


def _install_ntff_shim():
    """Provide antenv.axon_hooks if the image lacks it (trace=True path).

    Replicates trn_agent_boot's ctypes NTFF hook against libaxon_pjrt.so.
    """
    import sys

    if "antenv.axon_hooks" in sys.modules:
        return
    try:
        import antenv.axon_hooks  # noqa: F401
        return
    except ImportError:
        pass
    import contextlib
    import ctypes
    import types

    so_path = "/opt/axon/libaxon_pjrt.so"
    try:
        lib = ctypes.CDLL(so_path)
    except OSError:
        return
    if not hasattr(lib, "axon_start_nrt_profile"):
        return
    lib.axon_start_nrt_profile.argtypes = [
        ctypes.POINTER(ctypes.c_int64),
        ctypes.c_size_t,
    ]
    lib.axon_start_nrt_profile.restype = ctypes.c_int64
    lib.axon_stop_nrt_profile.argtypes = [ctypes.c_char_p]
    lib.axon_stop_nrt_profile.restype = ctypes.c_int64

    @contextlib.contextmanager
    def _hook(output_dir, device_ids):
        import jax

        jax.devices()
        if device_ids:
            ids = (ctypes.c_int64 * len(device_ids))(*device_ids)
            rc = lib.axon_start_nrt_profile(ids, len(device_ids))
        else:
            rc = lib.axon_start_nrt_profile(None, 0)
        if rc != 0:
            raise RuntimeError(f"axon_start_nrt_profile rc={rc}")
        try:
            yield
        finally:
            n = lib.axon_stop_nrt_profile(str(output_dir).encode())
            print(f"ntff profile: {n} file(s) written to {output_dir}")

    mod = types.ModuleType("antenv.axon_hooks")
    mod.get_axon_ntff_profile_hook = lambda: _hook
    mod.set_axon_ntff_profile_hook = lambda h: None
    sys.modules["antenv.axon_hooks"] = mod


def run(inputs, smooth, trace=False, **trace_kwargs):
    """Run on 8 cores; returns (y_full, BassKernelResults)."""
    if trace:
        _install_ntff_shim()
    nc = _get_nc()
    in_maps = _prep_in_maps(inputs, smooth)
    res = run_bass_kernel_spmd(
        nc, in_maps, list(range(NCORES)), trace=trace, **trace_kwargs
    )
    yT = np.concatenate([res.results[i]["y"] for i in range(NCORES)], axis=0)
    y = np.ascontiguousarray(yT.astype(np.float32).transpose(0, 2, 1))
    return y, res


def kernel(inputs, smooth):
    y, _ = run(inputs, smooth)
    return y


# revision 5
# speedup vs baseline: 1.4762x; 1.0041x over previous
"""Trainium2 Bass kernel: per-channel exponential moving average.

  a_t = k*x_t + (1-k)*a_{t-1},  a_{-1} = x_0   (per batch, per channel)

Full inputs: x [16, 8000, 512] f32, smooth [512] f32. Output [16, 8000, 512].

Strategy (8 NeuronCores, data-parallel over batch, 2 batches/core), with a
phase-decomposed scan that spreads the recurrence over all four engines:

  - Host pre-scales u = k*x, transposes to channel-major and deinterleaves
    time by R=8 phases: U[b, c, m, q] = u[b, q*R+m, c], fp16.
  - PE: block sums z[c,q] = sum_m d_c^(R-1-m) * U_m[c,q] via 8 accumulating
    matmuls with diagonal stationaries diag(d^pow) into PSUM (per-partition
    scale-and-add, which the PE does at 1 col/cycle in fp16).
  - DVE: tensor_tensor_scan only over the R-decimated series
    A[q] = d^R * A[q-1] + z[q]  (T/R elements per channel instead of T).
  - Recon chain per phase i: out_i = d * out_{i-1} + U_i, out_{-1} = Ashift:
    ACT does the scale (activation Copy with per-partition scale), DVE does
    the add (tensor_tensor, 2x fp16 mode).
  - fp16 I/O halves DMA bytes; d/d^R/x0 stay f32 (scan state is f32).
  - Host re-interleaves the fp16 output phases and upcasts to f32.
"""
import numpy as np
from contextlib import ExitStack

import concourse.bass as bass
from concourse import bacc, masks, mybir
import concourse.tile as tile
from concourse.bass_utils import run_bass_kernel_spmd

B, T, C = 16, 8000, 512
NCORES = 8
B_LOC = B // NCORES  # batches per core
P = 128
CG = C // P          # channel groups (4)
R = 8                # phase decimation factor
Q = T // R           # decimated length (1000)
QH = Q // 2          # psum-bank-sized chunk (500 f32 <= 512/bank)
F32 = mybir.dt.float32
F16 = mybir.dt.float16

_CACHED_NC = None


def _build_nc():
    nc = bacc.Bacc(None, target_bir_lowering=False)
    x = nc.declare_dram_parameter("x", [B_LOC, C, R, Q], F16, isOutput=False)
    dcol = nc.declare_dram_parameter("dcol", [P, CG], F32, isOutput=False)
    dR_pc = nc.declare_dram_parameter("dR_pc", [P, CG], F32, isOutput=False)
    dpow = nc.declare_dram_parameter("dpow", [P, CG, R], F32, isOutput=False)
    x0t = nc.declare_dram_parameter("x0t", [P, CG, B_LOC, 1], F32, isOutput=False)
    y = nc.declare_dram_parameter("y", [B_LOC, C, R, Q], F16, isOutput=True)

    with tile.TileContext(nc) as tc, ExitStack() as ctx:
        singles = ctx.enter_context(tc.tile_pool(name="singles", bufs=1))
        inpool = ctx.enter_context(tc.tile_pool(name="inpool", bufs=2))
        outpool = ctx.enter_context(tc.tile_pool(name="outpool", bufs=2))
        apool = ctx.enter_context(tc.tile_pool(name="apool", bufs=2))
        tmppool = ctx.enter_context(tc.tile_pool(name="tmppool", bufs=2))
        zpool = ctx.enter_context(tc.tile_pool(name="zpool", bufs=2, space="PSUM"))

        dcol_sb = singles.tile([P, CG], F32)
        nc.sync.dma_start(out=dcol_sb[:], in_=dcol[:])
        dR_sb = singles.tile([P, CG], F32)
        nc.sync.dma_start(out=dR_sb[:], in_=dR_pc[:])
        dpow_sb = singles.tile([P, CG, R], F32)
        nc.sync.dma_start(out=dpow_sb[:], in_=dpow[:])
        x0_sb = singles.tile([P, CG, B_LOC, 1], F32)
        nc.sync.dma_start(out=x0_sb[:], in_=x0t[:])

        ident = singles.tile([P, P], F32)
        masks.make_identity(nc, ident[:])
        diag = singles.tile([P, CG, R, P], F16)
        for cg in range(CG):
            for m in range(R):
                nc.vector.tensor_scalar(
                    diag[:, cg, m, :], ident[:],
                    dpow_sb[:, cg, m : m + 1], None,
                    mybir.AluOpType.mult,
                )
        ones = singles.tile([P, QH], F32)
        nc.vector.memset(ones[:], 1.0)
        dRbc = singles.tile([P, CG, QH], F32)
        for cg in range(CG):
            nc.scalar.activation(
                dRbc[:, cg, :], ones[:],
                mybir.ActivationFunctionType.Copy,
                scale=dR_sb[:, cg : cg + 1],
            )

        for cg in range(CG):
            cs = slice(cg * P, (cg + 1) * P)
            xin = inpool.tile([P, B_LOC, R, Q], F16, tag="xin", name="xin")
            for b in range(B_LOC):
                nc.gpsimd.dma_start(out=xin[:, b, :, :], in_=x[b, cs, :, :])

            # z[:, b, h, :QH] accumulates sum_m diag(d^{R-1-m}) @ U_m
            z = zpool.tile([P, B_LOC, 2, 512], F32, tag="z", name="z")
            for m in range(R):
                for b in range(B_LOC):
                    for h in range(2):
                        nc.tensor.matmul(
                            z[:, b, h, 0:QH],
                            diag[:, cg, m, :],
                            xin[:, b, m, h * QH : (h + 1) * QH],
                            start=(m == 0),
                            stop=(m == R - 1),
                        )

            # A[:, b, 0] = x0 carry slot; scan fills A[:, b, 1:1+Q]
            A = apool.tile([P, B_LOC, 1 + Q], F16, tag="A", name="A")
            nc.scalar.activation(
                A[:, :, 0:1], x0_sb[:, cg, :, :],
                mybir.ActivationFunctionType.Copy,
            )
            for b in range(B_LOC):
                for h in range(2):
                    init = (
                        x0_sb[:, cg, b, :]
                        if h == 0
                        else A[:, b, QH : QH + 1]
                    )
                    nc.vector.tensor_tensor_scan(
                        A[:, b, 1 + h * QH : 1 + (h + 1) * QH],
                        dRbc[:, cg, :],
                        z[:, b, h, 0:QH],
                        init,
                        mybir.AluOpType.mult,
                        mybir.AluOpType.add,
                    )

            # recon: out_i = d*out_{i-1} + U_i, out_{-1} = A shifted
            out = outpool.tile([P, B_LOC, R, Q], F16, tag="out", name="out")
            for i in range(R):
                prev = A[:, :, 0:Q] if i == 0 else out[:, :, i - 1, :]
                tmp = tmppool.tile([P, B_LOC, Q], F16, tag="tmp", name="tmp")
                nc.scalar.activation(
                    tmp[:], prev,
                    mybir.ActivationFunctionType.Copy,
                    scale=dcol_sb[:, cg : cg + 1],
                )
                nc.vector.tensor_tensor(
                    out[:, :, i, :], tmp[:], xin[:, :, i, :],
                    mybir.AluOpType.add,
                )
            for b in range(B_LOC):
                nc.gpsimd.dma_start(out=y[b, cs, :, :], in_=out[:, b, :, :])
    nc.compile()
    return nc


def _get_nc():
    global _CACHED_NC
    if _CACHED_NC is None:
        _CACHED_NC = _build_nc()
    return _CACHED_NC


def _prep_in_maps(inputs, smooth):
    import ml_dtypes

    x = np.asarray(inputs, dtype=np.float32)
    sm = np.asarray(smooth, dtype=np.float32)
    k = np.clip(sm, 0.0, 1.0).astype(np.float32)
    d = (1.0 - k).astype(np.float32)
    # U[b, c, m, q] = (k*x)[b, q*R+m, c], fp16
    kxT = (x * k[None, None, :]).transpose(0, 2, 1)  # [B, C, T]
    U = np.ascontiguousarray(
        kxT.reshape(B, C, Q, R).transpose(0, 1, 3, 2)
    ).astype(ml_dtypes.float16 if hasattr(ml_dtypes, "float16") else np.float16)
    dcol = np.ascontiguousarray(d.reshape(CG, P).T)                    # [P, CG]
    dR = np.ascontiguousarray((d.astype(np.float64) ** R).astype(np.float32).reshape(CG, P).T)
    # dpow[p, cg, m] = d^(R-1-m)
    pw = np.stack([d.astype(np.float64) ** (R - 1 - m) for m in range(R)], axis=1)
    dpow = np.ascontiguousarray(
        pw.astype(np.float32).reshape(CG, P, R).transpose(1, 0, 2)
    )
    # x0t[p, cg, b, 1] = x[b, 0, cg*P + p]
    x0 = x[:, 0, :].T.reshape(CG, P, B).transpose(1, 0, 2)[..., None]
    return [
        {
            "x": np.ascontiguousarray(U[i * B_LOC : (i + 1) * B_LOC]),
            "dcol": dcol,
            "dR_pc": dR,
            "dpow": dpow,
            "x0t": np.ascontiguousarray(x0[:, :, i * B_LOC : (i + 1) * B_LOC, :]),
        }
        for i in range(NCORES)
    ]


def _install_ntff_shim():
    """Provide antenv.axon_hooks if the image lacks it (trace=True path).

    Replicates trn_agent_boot's ctypes NTFF hook against libaxon_pjrt.so.
    """
    import sys

    if "antenv.axon_hooks" in sys.modules:
        return
    try:
        import antenv.axon_hooks  # noqa: F401
        return
    except ImportError:
        pass
    import contextlib
    import ctypes
    import types

    so_path = "/opt/axon/libaxon_pjrt.so"
    try:
        lib = ctypes.CDLL(so_path)
    except OSError:
        return
    if not hasattr(lib, "axon_start_nrt_profile"):
        return
    lib.axon_start_nrt_profile.argtypes = [
        ctypes.POINTER(ctypes.c_int64),
        ctypes.c_size_t,
    ]
    lib.axon_start_nrt_profile.restype = ctypes.c_int64
    lib.axon_stop_nrt_profile.argtypes = [ctypes.c_char_p]
    lib.axon_stop_nrt_profile.restype = ctypes.c_int64

    @contextlib.contextmanager
    def _hook(output_dir, device_ids):
        import jax

        jax.devices()
        if device_ids:
            ids = (ctypes.c_int64 * len(device_ids))(*device_ids)
            rc = lib.axon_start_nrt_profile(ids, len(device_ids))
        else:
            rc = lib.axon_start_nrt_profile(None, 0)
        if rc != 0:
            raise RuntimeError(f"axon_start_nrt_profile rc={rc}")
        try:
            yield
        finally:
            n = lib.axon_stop_nrt_profile(str(output_dir).encode())
            print(f"ntff profile: {n} file(s) written to {output_dir}")

    mod = types.ModuleType("antenv.axon_hooks")
    mod.get_axon_ntff_profile_hook = lambda: _hook
    mod.set_axon_ntff_profile_hook = lambda h: None
    sys.modules["antenv.axon_hooks"] = mod


def run(inputs, smooth, trace=False, **trace_kwargs):
    """Run on 8 cores; returns (y_full, BassKernelResults)."""
    if trace:
        _install_ntff_shim()
    nc = _get_nc()
    in_maps = _prep_in_maps(inputs, smooth)
    res = run_bass_kernel_spmd(
        nc, in_maps, list(range(NCORES)), trace=trace, **trace_kwargs
    )
    yp = np.concatenate([res.results[i]["y"] for i in range(NCORES)], axis=0)
    # y[b, t, c] with t = q*R + m  <-  yp[b, c, m, q]
    yf = yp.astype(np.float32).transpose(0, 3, 2, 1).reshape(B, T, C)
    return np.ascontiguousarray(yf), res


def kernel(inputs, smooth):
    y, _ = run(inputs, smooth)
    return y


# revision 6
# speedup vs baseline: 1.5500x; 1.0500x over previous
"""Trainium2 Bass kernel: per-channel exponential moving average.

  a_t = k*x_t + (1-k)*a_{t-1},  a_{-1} = x_0   (per batch, per channel)

Full inputs: x [16, 8000, 512] f32, smooth [512] f32. Output [16, 8000, 512].

Strategy (8 NeuronCores, data-parallel over batch, 2 batches/core), with a
phase-decomposed scan that spreads the recurrence over all four engines:

  - Host pre-scales u = k*x, transposes to channel-major and deinterleaves
    time by R=8 phases: U[b, c, m, q] = u[b, q*R+m, c], fp16.
  - PE: block sums z[c,q] = sum_m d_c^(R-1-m) * U_m[c,q] via 8 accumulating
    matmuls with diagonal stationaries diag(d^pow) into PSUM (per-partition
    scale-and-add, which the PE does at 1 col/cycle in fp16).
  - DVE: tensor_tensor_scan only over the R-decimated series
    A[q] = d^R * A[q-1] + z[q]  (T/R elements per channel instead of T).
  - Recon chain per phase i: out_i = d * out_{i-1} + U_i, out_{-1} = Ashift:
    ACT does the scale (activation Copy with per-partition scale), DVE does
    the add (tensor_tensor, 2x fp16 mode).
  - fp16 I/O halves DMA bytes; d/d^R/x0 stay f32 (scan state is f32).
  - Host re-interleaves the fp16 output phases and upcasts to f32.
"""
import numpy as np
from contextlib import ExitStack

import concourse.bass as bass
from concourse import bacc, masks, mybir
import concourse.tile as tile
from concourse.bass_utils import run_bass_kernel_spmd

B, T, C = 16, 8000, 512
NCORES = 8
B_LOC = B // NCORES  # batches per core
P = 128
CG = C // P          # channel groups (4)
R = 8                # phase decimation factor
Q = T // R           # decimated length (1000)
QH = Q // 2          # psum-bank-sized chunk (500 f32 <= 512/bank)
F32 = mybir.dt.float32
F16 = mybir.dt.float16

_CACHED_NC = None


def _build_nc():
    nc = bacc.Bacc(None, target_bir_lowering=False)
    x = nc.declare_dram_parameter("x", [B_LOC, C, R, Q], F16, isOutput=False)
    dcol = nc.declare_dram_parameter("dcol", [P, CG], F32, isOutput=False)
    dR_pc = nc.declare_dram_parameter("dR_pc", [P, CG], F32, isOutput=False)
    dpow = nc.declare_dram_parameter("dpow", [P, CG, R], F32, isOutput=False)
    x0t = nc.declare_dram_parameter("x0t", [P, CG, B_LOC, 1], F32, isOutput=False)
    y = nc.declare_dram_parameter("y", [B_LOC, C, R, Q], F16, isOutput=True)

    with tile.TileContext(nc) as tc, ExitStack() as ctx:
        singles = ctx.enter_context(tc.tile_pool(name="singles", bufs=1))
        inpool = ctx.enter_context(tc.tile_pool(name="inpool", bufs=2))
        outpool = ctx.enter_context(tc.tile_pool(name="outpool", bufs=2))
        apool = ctx.enter_context(tc.tile_pool(name="apool", bufs=2))
        tmppool = ctx.enter_context(tc.tile_pool(name="tmppool", bufs=2))
        zpool = ctx.enter_context(tc.tile_pool(name="zpool", bufs=2, space="PSUM"))

        dcol_sb = singles.tile([P, CG], F32)
        nc.sync.dma_start(out=dcol_sb[:], in_=dcol[:])
        dR_sb = singles.tile([P, CG], F32)
        nc.sync.dma_start(out=dR_sb[:], in_=dR_pc[:])
        dpow_sb = singles.tile([P, CG, R], F32)
        nc.sync.dma_start(out=dpow_sb[:], in_=dpow[:])
        x0_sb = singles.tile([P, CG, B_LOC, 1], F32)
        nc.sync.dma_start(out=x0_sb[:], in_=x0t[:])

        ident = singles.tile([P, P], F32)
        masks.make_identity(nc, ident[:])
        diag = singles.tile([P, CG, R, P], F16)
        for cg in range(CG):
            for m in range(R):
                nc.vector.tensor_scalar(
                    diag[:, cg, m, :], ident[:],
                    dpow_sb[:, cg, m : m + 1], None,
                    mybir.AluOpType.mult,
                )
        ones = singles.tile([P, QH], F32)
        nc.vector.memset(ones[:], 1.0)
        dRbc = singles.tile([P, CG, QH], F32)
        for cg in range(CG):
            nc.scalar.activation(
                dRbc[:, cg, :], ones[:],
                mybir.ActivationFunctionType.Copy,
                scale=dR_sb[:, cg : cg + 1],
            )

        for cg in range(CG):
            cs = slice(cg * P, (cg + 1) * P)
            xin = inpool.tile([P, B_LOC, R, Q], F16, tag="xin", name="xin")
            for b in range(B_LOC):
                nc.gpsimd.dma_start(out=xin[:, b, :, :], in_=x[b, cs, :, :])

            # z[:, b, h, :QH] accumulates sum_m diag(d^{R-1-m}) @ U_m
            z = zpool.tile([P, B_LOC, 2, 512], F32, tag="z", name="z")
            for m in range(R):
                for b in range(B_LOC):
                    for h in range(2):
                        nc.tensor.matmul(
                            z[:, b, h, 0:QH],
                            diag[:, cg, m, :],
                            xin[:, b, m, h * QH : (h + 1) * QH],
                            start=(m == 0),
                            stop=(m == R - 1),
                        )

            # A[:, b, 0] = x0 carry slot; scan fills A[:, b, 1:1+Q]
            A = apool.tile([P, B_LOC, 1 + Q], F16, tag="A", name="A")
            nc.scalar.activation(
                A[:, :, 0:1], x0_sb[:, cg, :, :],
                mybir.ActivationFunctionType.Copy,
            )
            for b in range(B_LOC):
                for h in range(2):
                    init = (
                        x0_sb[:, cg, b, :]
                        if h == 0
                        else A[:, b, QH : QH + 1]
                    )
                    nc.vector.tensor_tensor_scan(
                        A[:, b, 1 + h * QH : 1 + (h + 1) * QH],
                        dRbc[:, cg, :],
                        z[:, b, h, 0:QH],
                        init,
                        mybir.AluOpType.mult,
                        mybir.AluOpType.add,
                    )

            # recon: out_i = d*out_{i-1} + U_i, out_{-1} = A shifted
            out = outpool.tile([P, B_LOC, R, Q], F16, tag="out", name="out")
            for i in range(R):
                prev = A[:, :, 0:Q] if i == 0 else out[:, :, i - 1, :]
                tmp = tmppool.tile([P, B_LOC, Q], F16, tag="tmp", name="tmp")
                nc.vector.tensor_scalar(
                    tmp[:], prev,
                    dcol_sb[:, cg : cg + 1], None,
                    mybir.AluOpType.mult,
                )
                nc.vector.tensor_tensor(
                    out[:, :, i, :], tmp[:], xin[:, :, i, :],
                    mybir.AluOpType.add,
                )
            for b in range(B_LOC):
                nc.gpsimd.dma_start(out=y[b, cs, :, :], in_=out[:, b, :, :])
    nc.compile()
    return nc


def _get_nc():
    global _CACHED_NC
    if _CACHED_NC is None:
        _CACHED_NC = _build_nc()
    return _CACHED_NC


def _prep_in_maps(inputs, smooth):
    import ml_dtypes

    x = np.asarray(inputs, dtype=np.float32)
    sm = np.asarray(smooth, dtype=np.float32)
    k = np.clip(sm, 0.0, 1.0).astype(np.float32)
    d = (1.0 - k).astype(np.float32)
    # U[b, c, m, q] = (k*x)[b, q*R+m, c], fp16
    kxT = (x * k[None, None, :]).transpose(0, 2, 1)  # [B, C, T]
    U = np.ascontiguousarray(
        kxT.reshape(B, C, Q, R).transpose(0, 1, 3, 2)
    ).astype(ml_dtypes.float16 if hasattr(ml_dtypes, "float16") else np.float16)
    dcol = np.ascontiguousarray(d.reshape(CG, P).T)                    # [P, CG]
    dR = np.ascontiguousarray((d.astype(np.float64) ** R).astype(np.float32).reshape(CG, P).T)
    # dpow[p, cg, m] = d^(R-1-m)
    pw = np.stack([d.astype(np.float64) ** (R - 1 - m) for m in range(R)], axis=1)
    dpow = np.ascontiguousarray(
        pw.astype(np.float32).reshape(CG, P, R).transpose(1, 0, 2)
    )
    # x0t[p, cg, b, 1] = x[b, 0, cg*P + p]
    x0 = x[:, 0, :].T.reshape(CG, P, B).transpose(1, 0, 2)[..., None]
    return [
        {
            "x": np.ascontiguousarray(U[i * B_LOC : (i + 1) * B_LOC]),
            "dcol": dcol,
            "dR_pc": dR,
            "dpow": dpow,
            "x0t": np.ascontiguousarray(x0[:, :, i * B_LOC : (i + 1) * B_LOC, :]),
        }
        for i in range(NCORES)
    ]


def _install_ntff_shim():
    """Provide antenv.axon_hooks if the image lacks it (trace=True path).

    Replicates trn_agent_boot's ctypes NTFF hook against libaxon_pjrt.so.
    """
    import sys

    if "antenv.axon_hooks" in sys.modules:
        return
    try:
        import antenv.axon_hooks  # noqa: F401
        return
    except ImportError:
        pass
    import contextlib
    import ctypes
    import types

    so_path = "/opt/axon/libaxon_pjrt.so"
    try:
        lib = ctypes.CDLL(so_path)
    except OSError:
        return
    if not hasattr(lib, "axon_start_nrt_profile"):
        return
    lib.axon_start_nrt_profile.argtypes = [
        ctypes.POINTER(ctypes.c_int64),
        ctypes.c_size_t,
    ]
    lib.axon_start_nrt_profile.restype = ctypes.c_int64
    lib.axon_stop_nrt_profile.argtypes = [ctypes.c_char_p]
    lib.axon_stop_nrt_profile.restype = ctypes.c_int64

    @contextlib.contextmanager
    def _hook(output_dir, device_ids):
        import jax

        jax.devices()
        if device_ids:
            ids = (ctypes.c_int64 * len(device_ids))(*device_ids)
            rc = lib.axon_start_nrt_profile(ids, len(device_ids))
        else:
            rc = lib.axon_start_nrt_profile(None, 0)
        if rc != 0:
            raise RuntimeError(f"axon_start_nrt_profile rc={rc}")
        try:
            yield
        finally:
            n = lib.axon_stop_nrt_profile(str(output_dir).encode())
            print(f"ntff profile: {n} file(s) written to {output_dir}")

    mod = types.ModuleType("antenv.axon_hooks")
    mod.get_axon_ntff_profile_hook = lambda: _hook
    mod.set_axon_ntff_profile_hook = lambda h: None
    sys.modules["antenv.axon_hooks"] = mod


def run(inputs, smooth, trace=False, **trace_kwargs):
    """Run on 8 cores; returns (y_full, BassKernelResults)."""
    if trace:
        _install_ntff_shim()
    nc = _get_nc()
    in_maps = _prep_in_maps(inputs, smooth)
    res = run_bass_kernel_spmd(
        nc, in_maps, list(range(NCORES)), trace=trace, **trace_kwargs
    )
    yp = np.concatenate([res.results[i]["y"] for i in range(NCORES)], axis=0)
    # y[b, t, c] with t = q*R + m  <-  yp[b, c, m, q]
    yf = yp.astype(np.float32).transpose(0, 3, 2, 1).reshape(B, T, C)
    return np.ascontiguousarray(yf), res


def kernel(inputs, smooth):
    y, _ = run(inputs, smooth)
    return y


# revision 7
# speedup vs baseline: 1.8591x; 1.1994x over previous
"""Trainium2 Bass kernel: per-channel exponential moving average.

  a_t = k*x_t + (1-k)*a_{t-1},  a_{-1} = x_0   (per batch, per channel)

Full inputs: x [16, 8000, 512] f32, smooth [512] f32. Output [16, 8000, 512].

Strategy (8 NeuronCores, data-parallel over batch, 2 batches/core), with a
phase-decomposed scan that spreads the recurrence over all four engines:

  - Host pre-scales u = k*x, transposes to channel-major and deinterleaves
    time by R=8 phases, fp16, laid out [C, NU, B_LOC, R, QU] so each
    (channel-group, q-chunk) unit is one DMA call with 16KB/partition
    contiguous descriptors.
  - PE: block sums z[c,q] = sum_m d_c^(R-1-m) * U_m[c,q] via 8 accumulating
    matmuls with host-built diagonal stationaries diag(d^pow) into PSUM
    (per-partition scale-and-add at 1 col/cycle fp16).
  - DVE: tensor_tensor_scan only over the R-decimated series
    A[q] = d^R * A[q-1] + z[q]  (T/R elements per channel instead of T).
  - Recon chain per phase i: out_i = d * out_{i-1} + U_i, out_{-1} = Ashift:
    ACT does the scale (activation Copy with per-partition scale), DVE the
    add (tensor_tensor, 2x fp16 mode). Units are processed in software-
    interleaved pairs so the cross-engine chain never head-blocks either
    engine's in-order stream.
  - Host re-interleaves the fp16 output phases and upcasts to f32.
"""
import numpy as np
from contextlib import ExitStack

import concourse.bass as bass
from concourse import bacc, mybir
import concourse.tile as tile
from concourse.bass_utils import run_bass_kernel_spmd

B, T, C = 16, 8000, 512
NCORES = 8
B_LOC = B // NCORES  # batches per core
P = 128
CG = C // P          # channel groups (4)
R = 8                # phase decimation factor
Q = T // R           # decimated length (1000)
NU = 2               # q-chunks per channel group
QU = Q // NU         # 500 (fits one psum bank as f32)
F32 = mybir.dt.float32
F16 = mybir.dt.float16

_CACHED_NC = None


def _build_nc():
    nc = bacc.Bacc(None, target_bir_lowering=False)
    x = nc.declare_dram_parameter("x", [C, NU, B_LOC, R, QU], F16, isOutput=False)
    diag_d = nc.declare_dram_parameter("diag_d", [P, CG, R, P], F16, isOutput=False)
    dRbc_d = nc.declare_dram_parameter("dRbc_d", [P, CG, QU], F32, isOutput=False)
    dcol = nc.declare_dram_parameter("dcol", [P, CG], F32, isOutput=False)
    x0t = nc.declare_dram_parameter("x0t", [P, CG, B_LOC, 1], F32, isOutput=False)
    y = nc.declare_dram_parameter("y", [C, NU, B_LOC, R, QU], F16, isOutput=True)

    with tile.TileContext(nc) as tc, ExitStack() as ctx:
        singles = ctx.enter_context(tc.tile_pool(name="singles", bufs=1))
        inpool = ctx.enter_context(tc.tile_pool(name="inpool", bufs=5))
        outpool = ctx.enter_context(tc.tile_pool(name="outpool", bufs=3))
        apool = ctx.enter_context(tc.tile_pool(name="apool", bufs=2))
        tmppool = ctx.enter_context(tc.tile_pool(name="tmppool", bufs=4))
        zpool = ctx.enter_context(tc.tile_pool(name="zpool", bufs=4, space="PSUM"))

        diag = singles.tile([P, CG, R, P], F16)
        nc.gpsimd.dma_start(out=diag[:], in_=diag_d[:])
        dRbc = singles.tile([P, CG, QU], F32)
        nc.gpsimd.dma_start(out=dRbc[:], in_=dRbc_d[:])
        dcol_sb = singles.tile([P, CG], F32)
        nc.sync.dma_start(out=dcol_sb[:], in_=dcol[:])
        x0_sb = singles.tile([P, CG, B_LOC, 1], F32)
        nc.sync.dma_start(out=x0_sb[:], in_=x0t[:])

        units = [(cg, qc) for qc in range(NU) for cg in range(CG)]
        prev_A = {}

        def stage_front(cg, qc):
            """DMA in, PE z-accumulation, A carry slot, scans. Returns tiles."""
            cs = slice(cg * P, (cg + 1) * P)
            xin = inpool.tile([P, B_LOC, R, QU], F16, tag="xin", name="xin")
            nc.gpsimd.dma_start(out=xin[:], in_=x[cs, qc, :, :, :])
            z = zpool.tile([P, B_LOC, 512], F32, tag="z", name="z")
            for m in range(R):
                for b in range(B_LOC):
                    nc.tensor.matmul(
                        z[:, b, 0:QU],
                        diag[:, cg, m, :],
                        xin[:, b, m, :],
                        start=(m == 0),
                        stop=(m == R - 1),
                    )
            A = apool.tile([P, B_LOC, 1 + QU], F16, tag=f"A{cg}", name=f"A{cg}")
            carry = (
                x0_sb[:, cg, :, :] if qc == 0 else prev_A[cg][:, :, QU : QU + 1]
            )
            nc.scalar.activation(
                A[:, :, 0:1], carry, mybir.ActivationFunctionType.Copy
            )
            for b in range(B_LOC):
                init = (
                    x0_sb[:, cg, b, :]
                    if qc == 0
                    else prev_A[cg][:, b, QU : QU + 1]
                )
                nc.vector.tensor_tensor_scan(
                    A[:, b, 1 : 1 + QU],
                    dRbc[:, cg, :],
                    z[:, b, 0:QU],
                    init,
                    mybir.AluOpType.mult,
                    mybir.AluOpType.add,
                )
            prev_A[cg] = A
            out = outpool.tile([P, B_LOC, R, QU], F16, tag="out", name="out")
            return cs, xin, A, out

        def recon_phase(st, cg, i):
            cs, xin, A, out = st
            prev = A[:, :, 0:QU] if i == 0 else out[:, :, i - 1, :]
            tmp = tmppool.tile([P, B_LOC, QU], F16, tag="tmp", name="tmp")
            nc.scalar.activation(
                tmp[:], prev,
                mybir.ActivationFunctionType.Copy,
                scale=dcol_sb[:, cg : cg + 1],
            )
            nc.vector.tensor_tensor(
                out[:, :, i, :], tmp[:], xin[:, :, i, :],
                mybir.AluOpType.add,
            )

        # process units in software-interleaved pairs
        for j in range(0, len(units), 2):
            (cgA, qcA), (cgB, qcB) = units[j], units[j + 1]
            stA = stage_front(cgA, qcA)
            stB = stage_front(cgB, qcB)
            for i in range(R):
                recon_phase(stA, cgA, i)
                recon_phase(stB, cgB, i)
            for st, cg, qc in ((stA, cgA, qcA), (stB, cgB, qcB)):
                cs, xin, A, out = st
                nc.gpsimd.dma_start(out=y[cs, qc, :, :, :], in_=out[:])
    nc.compile()
    return nc


def _get_nc():
    global _CACHED_NC
    if _CACHED_NC is None:
        _CACHED_NC = _build_nc()
    return _CACHED_NC


def _prep_in_maps(inputs, smooth):
    import ml_dtypes

    f16 = np.dtype("float16")
    x = np.asarray(inputs, dtype=np.float32)
    sm = np.asarray(smooth, dtype=np.float32)
    k = np.clip(sm, 0.0, 1.0).astype(np.float32)
    d = (1.0 - k).astype(np.float32)
    # U[c, qc, b, m, ql] = (k*x)[b, (qc*QU+ql)*R + m, c]
    kxT = (x * k[None, None, :]).transpose(0, 2, 1)  # [B, C, T]
    U = np.ascontiguousarray(
        kxT.reshape(B, C, NU, QU, R).transpose(1, 2, 0, 4, 3)
    ).astype(f16)  # [C, NU, B, R, QU]
    dcol = np.ascontiguousarray(d.reshape(CG, P).T)  # [P, CG]
    d64 = d.astype(np.float64)
    # diag[p, cg, m, j] = d_c^(R-1-m) if j==p else 0, c = cg*P+p
    pw = np.stack([d64 ** (R - 1 - m) for m in range(R)], axis=1)  # [C, R]
    pw = pw.astype(np.float32).reshape(CG, P, R).transpose(1, 0, 2)  # [P, CG, R]
    diag = np.zeros((P, CG, R, P), dtype=f16)
    idx = np.arange(P)
    diag[idx, :, :, idx] = pw.astype(f16)
    dR = (d64 ** R).astype(np.float32).reshape(CG, P).T  # [P, CG]
    dRbc = np.ascontiguousarray(
        np.repeat(dR[:, :, None], QU, axis=2)
    )  # [P, CG, QU]
    x0 = x[:, 0, :].T.reshape(CG, P, B).transpose(1, 0, 2)[..., None]
    return [
        {
            "x": np.ascontiguousarray(U[:, :, i * B_LOC : (i + 1) * B_LOC]),
            "diag_d": diag,
            "dRbc_d": dRbc,
            "dcol": dcol,
            "x0t": np.ascontiguousarray(x0[:, :, i * B_LOC : (i + 1) * B_LOC, :]),
        }
        for i in range(NCORES)
    ]


def _install_ntff_shim():
    """Provide antenv.axon_hooks if the image lacks it (trace=True path).

    Replicates trn_agent_boot's ctypes NTFF hook against libaxon_pjrt.so.
    """
    import sys

    if "antenv.axon_hooks" in sys.modules:
        return
    try:
        import antenv.axon_hooks  # noqa: F401
        return
    except ImportError:
        pass
    import contextlib
    import ctypes
    import types

    so_path = "/opt/axon/libaxon_pjrt.so"
    try:
        lib = ctypes.CDLL(so_path)
    except OSError:
        return
    if not hasattr(lib, "axon_start_nrt_profile"):
        return
    lib.axon_start_nrt_profile.argtypes = [
        ctypes.POINTER(ctypes.c_int64),
        ctypes.c_size_t,
    ]
    lib.axon_start_nrt_profile.restype = ctypes.c_int64
    lib.axon_stop_nrt_profile.argtypes = [ctypes.c_char_p]
    lib.axon_stop_nrt_profile.restype = ctypes.c_int64

    @contextlib.contextmanager
    def _hook(output_dir, device_ids):
        import jax

        jax.devices()
        if device_ids:
            ids = (ctypes.c_int64 * len(device_ids))(*device_ids)
            rc = lib.axon_start_nrt_profile(ids, len(device_ids))
        else:
            rc = lib.axon_start_nrt_profile(None, 0)
        if rc != 0:
            raise RuntimeError(f"axon_start_nrt_profile rc={rc}")
        try:
            yield
        finally:
            n = lib.axon_stop_nrt_profile(str(output_dir).encode())
            print(f"ntff profile: {n} file(s) written to {output_dir}")

    mod = types.ModuleType("antenv.axon_hooks")
    mod.get_axon_ntff_profile_hook = lambda: _hook
    mod.set_axon_ntff_profile_hook = lambda h: None
    sys.modules["antenv.axon_hooks"] = mod


def run(inputs, smooth, trace=False, **trace_kwargs):
    """Run on 8 cores; returns (y_full, BassKernelResults)."""
    if trace:
        _install_ntff_shim()
    nc = _get_nc()
    in_maps = _prep_in_maps(inputs, smooth)
    res = run_bass_kernel_spmd(
        nc, in_maps, list(range(NCORES)), trace=trace, **trace_kwargs
    )
    # yp [C, NU, B_LOC, R, QU] per core; batch axis is dim 2
    yp = np.concatenate([res.results[i]["y"] for i in range(NCORES)], axis=2)
    # y[b, t, c] with t = (qc*QU + ql)*R + m
    yf = (
        yp.astype(np.float32).transpose(2, 1, 4, 3, 0).reshape(B, T, C)
    )
    return np.ascontiguousarray(yf), res


def kernel(inputs, smooth):
    y, _ = run(inputs, smooth)
    return y
